# revision 1
# baseline (speedup 1.0000x reference)
"""ParallelHyenaOperator Trainium2 kernel.

out = (irfft(rfft(u,2L) * rfft(k,2L))[:L] + u*d_bias) * x1,  u = x2*v, k = h*decay

Strategy: shard D=768 channels across 8 cores (96/core). Per channel, both
batches are packed into one complex FFT (z = u0 + i*u1); the conv theorem
gives y0 + i*y1 = ifft(fft(z) * fft(k)). The 16384-point FFT is a two-stage
radix-128 factorization where each stage is a 128x128 matmul on the tensor
engine (bf16), with pointwise twiddle/product stages on DVE and PSUM->SBUF
evacuation on the scalar engine. Pre/post gating stays fp32.

Note: all matmul operands are kept at base_partition 0 — an accumulating
matmul pair whose second operand sits at base partition 64 was observed to
hard-fault the device.
"""

import math
import os
import numpy as np
import ml_dtypes

B, D, L = 2, 768, 8192
NCORES = 8
DPC = int(os.environ.get("HYENA_DPC", D // NCORES))  # channels per core
NF = 2 * L                 # 16384 FFT size
G = 4                      # channels per group (batched free dim = 512)
NGROUPS = DPC // G
LOG_R_MIN, LOG_R_MAX = 0.0, 2.0

BF16 = ml_dtypes.bfloat16


def _make_consts():
    n2 = np.arange(64)
    n1 = np.arange(128)
    k1 = np.arange(128)
    k2 = np.arange(128)
    m = np.arange(64)

    Wc = np.exp(-2j * np.pi * np.outer(n2, k2) / 128)        # [64,128]
    T = np.exp(-2j * np.pi * np.outer(n1, k2) / NF)          # [128,128]
    W2 = np.exp(-2j * np.pi * np.outer(n1, k1) / 128)        # [128,128]
    Wcc = np.exp(+2j * np.pi * np.outer(k1, n1) / 128)       # [128,128]
    T2t = np.exp(+2j * np.pi * np.outer(k2, n1) / NF)        # [128,128] ([k2,n1])
    W2c = np.exp(+2j * np.pi * np.outer(k2, m) / 128) / NF   # [128,64]

    bf = lambda a: np.ascontiguousarray(a, dtype=np.float32).astype(BF16)
    c = {}
    c["wc_r"] = bf(Wc.real)          # [64,128]
    c["wc_i"] = bf(Wc.imag)
    c["wc_ni"] = bf(-Wc.imag)
    c["w2_r"] = bf(W2.real)
    c["w2_i"] = bf(W2.imag)
    c["w2_ni"] = bf(-W2.imag)
    c["wcc_r"] = bf(Wcc.real)
    c["wcc_i"] = bf(Wcc.imag)
    c["wcc_ni"] = bf(-Wcc.imag)
    # twiddles replicated G times along free dim
    c["t_r"] = bf(np.tile(T.real, (1, G)))
    c["t_i"] = bf(np.tile(T.imag, (1, G)))
    c["t2t_r"] = bf(np.tile(T2t.real, (1, G)))
    c["t2t_i"] = bf(np.tile(T2t.imag, (1, G)))
    # S2' weights [k2, n2] (64 cols each)
    c["w2c_r"] = bf(W2c.real)
    c["w2c_i"] = bf(W2c.imag)
    c["w2c_ni"] = bf(-W2c.imag)

    # decay = exp(-logspace(r)[d] * linspace(0,1,L)), module constant
    r = np.logspace(LOG_R_MIN, LOG_R_MAX, D).astype(np.float64)
    t = np.linspace(0.0, 1.0, L)
    decay = np.exp(-np.outer(r, t))
    c["_decay_full"] = np.ascontiguousarray(decay.astype(np.float32))
    return c


_CONSTS = _make_consts()
_NC_CACHE = {}

CONST_NAMES = ["wc_r", "wc_i", "wc_ni", "w2_r", "w2_i", "w2_ni",
               "wcc_r", "wcc_i", "wcc_ni", "t_r", "t_i", "t2t_r", "t2t_i",
               "w2c_r", "w2c_i", "w2c_ni"]


def _build_nc():
    import concourse.bacc as bacc
    import concourse.tile as tile
    from concourse import mybir

    dt = mybir.dt
    AF = mybir.AluOpType

    nc = bacc.Bacc("TRN2", target_bir_lowering=False, debug=False,
                   num_devices=NCORES)

    def din(name, shape, d):
        return nc.dram_tensor(name, shape, d, kind="ExternalInput").ap()

    x1d = din("x1s", [B, DPC, L], dt.float32)
    x2d = din("x2s", [B, DPC, L], dt.float32)
    vd = din("vs", [B, DPC, L], dt.float32)
    hd = din("hs", [DPC, L], dt.float32)
    dbd = din("db_bc", [128, DPC], dt.float32)
    decd = din("decays", [DPC, L], dt.float32)
    cc = {}
    for nm in CONST_NAMES:
        shp = list(_CONSTS[nm].shape)
        cc[nm] = din(nm, shp, dt.bfloat16)
    outd = nc.dram_tensor("out", [B, DPC, L], dt.float32,
                          kind="ExternalOutput").ap()

    FW = 128 * G  # group free width

    with tile.TileContext(nc, trace_sim=False) as tc:
        cpool = tc.alloc_tile_pool(name="consts", bufs=1)
        iopool = tc.alloc_tile_pool(name="io", bufs=2 * G)
        upool = tc.alloc_tile_pool(name="u", bufs=3 * G)
        gpool = tc.alloc_tile_pool(name="grp", bufs=2)
        pspool = tc.alloc_tile_pool(name="ps", bufs=7, space="PSUM")

        csb = {}
        for nm, ap in cc.items():
            t = cpool.tile(list(ap.shape), dt.bfloat16, tag=nm)
            nc.sync.dma_start(t[:], ap)
            csb[nm] = t
        dbt = cpool.tile([128, DPC], dt.float32, tag="dbt")
        nc.sync.dma_start(dbt[:], dbd)

        def cmul(eng, out_r, out_i, a_r, a_i, b_r, b_i, tmp_pool, fw):
            # (out_r + i*out_i) = (a_r + i*a_i) * (b_r + i*b_i), bf16
            m1 = tmp_pool.tile([128, fw], dt.bfloat16, tag="cm_m1")
            m2 = tmp_pool.tile([128, fw], dt.bfloat16, tag="cm_m2")
            eng.tensor_tensor(m1[:], a_r[:], b_r[:], AF.mult)
            eng.tensor_tensor(m2[:], a_i[:], b_i[:], AF.mult)
            eng.tensor_tensor(out_r[:], m1[:], m2[:], AF.subtract)
            m3 = tmp_pool.tile([128, fw], dt.bfloat16, tag="cm_m1")
            m4 = tmp_pool.tile([128, fw], dt.bfloat16, tag="cm_m2")
            eng.tensor_tensor(m3[:], a_r[:], b_i[:], AF.mult)
            eng.tensor_tensor(m4[:], a_i[:], b_r[:], AF.mult)
            eng.tensor_tensor(out_i[:], m3[:], m4[:], AF.add)

        for g in range(NGROUPS):
            chans = [g * G + j for j in range(G)]
            uts, x1ts = [], []
            # S1 psum accumulators, one [128,128] slice per channel
            z0r = pspool.tile([128, FW], dt.float32, tag="ps")
            z0i = pspool.tile([128, FW], dt.float32, tag="ps")
            zk0r = pspool.tile([128, FW], dt.float32, tag="ps")
            zk0i = pspool.tile([128, FW], dt.float32, tag="ps")
            for j, c in enumerate(chans):
                # [64, 256] layout: batch b occupies free cols [128b, 128b+128)
                x2t = iopool.tile([64, 256], dt.float32, tag="x2")
                vt = iopool.tile([64, 256], dt.float32, tag="v")
                x1t = iopool.tile([64, 256], dt.float32, tag="x1")
                ht = iopool.tile([64, 128], dt.float32, tag="h")
                dct = iopool.tile([64, 128], dt.float32, tag="dec")
                for bb in range(2):
                    fs = slice(128 * bb, 128 * (bb + 1))
                    nc.sync.dma_start(
                        x2t[:, fs],
                        x2d[bb, c, :].rearrange("(p q) -> p q", p=64))
                    nc.sync.dma_start(
                        vt[:, fs],
                        vd[bb, c, :].rearrange("(p q) -> p q", p=64))
                    nc.sync.dma_start(
                        x1t[:, fs],
                        x1d[bb, c, :].rearrange("(p q) -> p q", p=64))
                nc.sync.dma_start(
                    ht[:], hd[c, :].rearrange("(p q) -> p q", p=64))
                nc.sync.dma_start(
                    dct[:], decd[c, :].rearrange("(p q) -> p q", p=64))

                ut = upool.tile([64, 256], dt.float32, tag="u")
                nc.vector.tensor_tensor(ut[:], x2t[:], vt[:], AF.mult)
                zt = upool.tile([64, 256], dt.bfloat16, tag="z")
                nc.vector.tensor_copy(zt[:], ut[:])
                kbt = upool.tile([64, 128], dt.bfloat16, tag="kb")
                nc.vector.tensor_tensor(kbt[:], ht[:], dct[:], AF.mult)
                uts.append(ut)
                x1ts.append(x1t)

                sl = slice(j * 128, (j + 1) * 128)
                zr = zt[:, 0:128]
                zi = zt[:, 128:256]
                wr, wi, wni = csb["wc_r"], csb["wc_i"], csb["wc_ni"]
                # S1 (u): Z0[n1,k2] = sum_n2 z[n2,n1] * Wc[n2,k2]
                nc.tensor.matmul(z0r[:, sl], zr, wr[:], start=True, stop=False)
                nc.tensor.matmul(z0r[:, sl], zi, wni[:], start=False, stop=True)
                nc.tensor.matmul(z0i[:, sl], zr, wi[:], start=True, stop=False)
                nc.tensor.matmul(z0i[:, sl], zi, wr[:], start=False, stop=True)
                # S1 (k): real input
                nc.tensor.matmul(zk0r[:, sl], kbt[:], wr[:], start=True, stop=True)
                nc.tensor.matmul(zk0i[:, sl], kbt[:], wi[:], start=True, stop=True)

            # evacuate S1 psum -> bf16 sbuf (scalar engine)
            z0rb = gpool.tile([128, FW], dt.bfloat16, tag="z0rb")
            z0ib = gpool.tile([128, FW], dt.bfloat16, tag="z0ib")
            zk0rb = gpool.tile([128, FW], dt.bfloat16, tag="zk0rb")
            zk0ib = gpool.tile([128, FW], dt.bfloat16, tag="zk0ib")
            nc.scalar.copy(z0rb[:], z0r[:])
            nc.scalar.copy(z0ib[:], z0i[:])
            nc.scalar.copy(zk0rb[:], zk0r[:])
            nc.scalar.copy(zk0ib[:], zk0i[:])

            # forward twiddle
            z1r = gpool.tile([128, FW], dt.bfloat16, tag="z1r")
            z1i = gpool.tile([128, FW], dt.bfloat16, tag="z1i")
            cmul(nc.vector, z1r, z1i, z0rb, z0ib, csb["t_r"], csb["t_i"],
                 gpool, FW)
            zk1r = gpool.tile([128, FW], dt.bfloat16, tag="zk1r")
            zk1i = gpool.tile([128, FW], dt.bfloat16, tag="zk1i")
            cmul(nc.vector, zk1r, zk1i, zk0rb, zk0ib, csb["t_r"], csb["t_i"],
                 gpool, FW)

            # S2: P[k1,k2] = sum_n1 W2[n1,k1] * Z1[n1,k2], grouped N=512
            pzr = pspool.tile([128, FW], dt.float32, tag="ps")
            pzi = pspool.tile([128, FW], dt.float32, tag="ps")
            pkr = pspool.tile([128, FW], dt.float32, tag="ps")
            pki = pspool.tile([128, FW], dt.float32, tag="ps")
            w2r, w2i, w2ni = csb["w2_r"], csb["w2_i"], csb["w2_ni"]
            nc.tensor.matmul(pzr[:], w2r[:], z1r[:], start=True, stop=False)
            nc.tensor.matmul(pzi[:], w2i[:], z1r[:], start=True, stop=False)
            nc.tensor.matmul(pkr[:], w2r[:], zk1r[:], start=True, stop=False)
            nc.tensor.matmul(pki[:], w2i[:], zk1r[:], start=True, stop=False)
            nc.tensor.matmul(pzr[:], w2ni[:], z1i[:], start=False, stop=True)
            nc.tensor.matmul(pkr[:], w2ni[:], zk1i[:], start=False, stop=True)
            nc.tensor.matmul(pzi[:], w2r[:], z1i[:], start=False, stop=True)
            nc.tensor.matmul(pki[:], w2r[:], zk1i[:], start=False, stop=True)

            # evacuate P psum -> bf16
            pzrb = gpool.tile([128, FW], dt.bfloat16, tag="pzrb")
            pzib = gpool.tile([128, FW], dt.bfloat16, tag="pzib")
            pkrb = gpool.tile([128, FW], dt.bfloat16, tag="pkrb")
            pkib = gpool.tile([128, FW], dt.bfloat16, tag="pkib")
            nc.scalar.copy(pzrb[:], pzr[:])
            nc.scalar.copy(pzib[:], pzi[:])
            nc.scalar.copy(pkrb[:], pkr[:])
            nc.scalar.copy(pkib[:], pki[:])

            # spectral product
            pyr = gpool.tile([128, FW], dt.bfloat16, tag="pyr")
            pyi = gpool.tile([128, FW], dt.bfloat16, tag="pyi")
            cmul(nc.vector, pyr, pyi, pzrb, pzib, pkrb, pkib, gpool, FW)

            # S1': At[k2,n1] = sum_k1 P_y[k1,k2] * Wcc[k1,n1], per-channel lhsT
            atr = pspool.tile([128, FW], dt.float32, tag="ps")
            ati = pspool.tile([128, FW], dt.float32, tag="ps")
            for j in range(G):
                sl = slice(j * 128, (j + 1) * 128)
                pr = pyr[:, sl]
                pi = pyi[:, sl]
                nc.tensor.matmul(atr[:, sl], pr, csb["wcc_r"][:], start=True, stop=False)
                nc.tensor.matmul(ati[:, sl], pr, csb["wcc_i"][:], start=True, stop=False)
                nc.tensor.matmul(atr[:, sl], pi, csb["wcc_ni"][:], start=False, stop=True)
                nc.tensor.matmul(ati[:, sl], pi, csb["wcc_r"][:], start=False, stop=True)

            atrb = gpool.tile([128, FW], dt.bfloat16, tag="atrb")
            atib = gpool.tile([128, FW], dt.bfloat16, tag="atib")
            nc.scalar.copy(atrb[:], atr[:])
            nc.scalar.copy(atib[:], ati[:])

            # inverse twiddle
            btr = gpool.tile([128, FW], dt.bfloat16, tag="btr")
            bti = gpool.tile([128, FW], dt.bfloat16, tag="bti")
            cmul(nc.vector, btr, bti, atrb, atib, csb["t2t_r"], csb["t2t_i"],
                 gpool, FW)

            # S2': y_b[n2,n1] = Re/Im of sum_k2 W2c[k2,n2] * Bt[k2,n1], grouped
            yg0 = pspool.tile([64, FW], dt.float32, tag="ps")   # Re -> batch 0
            yg1 = pspool.tile([64, FW], dt.float32, tag="ps")   # Im -> batch 1
            nc.tensor.matmul(yg0[:], csb["w2c_r"][:], btr[:], start=True, stop=False)
            nc.tensor.matmul(yg1[:], csb["w2c_i"][:], btr[:], start=True, stop=False)
            nc.tensor.matmul(yg0[:], csb["w2c_ni"][:], bti[:], start=False, stop=True)
            nc.tensor.matmul(yg1[:], csb["w2c_r"][:], bti[:], start=False, stop=True)

            # post: out_b = (y_b + db*u_b) * x1_b   (fp32)
            for j, c in enumerate(chans):
                sl = slice(j * 128, (j + 1) * 128)
                for bb, yg in enumerate((yg0, yg1)):
                    fs = slice(128 * bb, 128 * (bb + 1))
                    tt = upool.tile([64, 128], dt.float32, tag="t")
                    nc.vector.scalar_tensor_tensor(
                        tt[:], uts[j][:, fs], dbt[0:64, c:c + 1], yg[:, sl],
                        AF.mult, AF.add)
                    ot = upool.tile([64, 128], dt.float32, tag="o")
                    nc.vector.tensor_tensor(ot[:], tt[:], x1ts[j][:, fs],
                                            AF.mult)
                    nc.sync.dma_start(
                        outd[bb, c, :].rearrange("(p q) -> p q", p=64), ot[:])

        for p in (pspool, gpool, upool, iopool, cpool):
            p.release()

    nc.compile()
    return nc


def _get_nc():
    if "nc" not in _NC_CACHE:
        _NC_CACHE["nc"] = _build_nc()
    return _NC_CACHE["nc"]


def make_in_maps(x1, x2, v, h, d_bias):
    c = _CONSTS
    in_maps = []
    for core in range(NCORES):
        sl = slice(core * DPC, (core + 1) * DPC)
        m = {
            "x1s": np.ascontiguousarray(x1[:, sl]),
            "x2s": np.ascontiguousarray(x2[:, sl]),
            "vs": np.ascontiguousarray(v[:, sl]),
            "hs": np.ascontiguousarray(h[sl]),
            "db_bc": np.ascontiguousarray(
                np.broadcast_to(d_bias[sl][None, :], (128, DPC))),
            "decays": np.ascontiguousarray(c["_decay_full"][sl]),
        }
        for nm in CONST_NAMES:
            m[nm] = c[nm]
        in_maps.append(m)
    return in_maps


def kernel(x1, x2, v, h, d_bias):
    from concourse import bass_utils

    x1 = np.ascontiguousarray(x1, dtype=np.float32)
    x2 = np.ascontiguousarray(x2, dtype=np.float32)
    v = np.ascontiguousarray(v, dtype=np.float32)
    h = np.ascontiguousarray(h, dtype=np.float32)
    d_bias = np.ascontiguousarray(d_bias, dtype=np.float32)

    nc = _get_nc()
    in_maps = make_in_maps(x1, x2, v, h, d_bias)
    res = bass_utils.run_bass_kernel_spmd(
        nc, in_maps, core_ids=list(range(NCORES)))
    out = np.concatenate([r["out"] for r in res.results], axis=1)
    return out.astype(np.float32)


if __name__ == "__main__":
    rng = np.random.default_rng(0)
    inputs = {
        "x1": rng.standard_normal((B, D, L)).astype(np.float32),
        "x2": rng.standard_normal((B, D, L)).astype(np.float32),
        "v": rng.standard_normal((B, D, L)).astype(np.float32),
        "h": (rng.standard_normal((D, L)) / math.sqrt(L) * 1e-5).astype(np.float32),
        "d_bias": rng.standard_normal(D).astype(np.float32),
    }
    out = kernel(**inputs)
    print(out.shape, out.dtype)



# revision 3
# speedup vs baseline: 1.5125x; 1.5125x over previous
"""ParallelHyenaOperator Trainium2 kernel (v2: pair-stacked slab design).

out = (irfft(rfft(u,2L) * rfft(k,2L))[:L] + u*d_bias) * x1,  u = x2*v, k = h*decay

Sharding: D=768 channels split across 8 cores (96/core), no collectives.
Per core, channels are paired (c, c+48) and stacked in SBUF partitions
(c -> rows 0:64, c+48 -> rows 64:128), 12 pairs per slab, 4 slabs.
Each 16384-point FFT is a two-stage radix-128 factorization on the tensor
engine; both batches are packed as one complex series (z = u_b0 + i*u_b1).
Stage-1 matmuls take the stacked pair as the stationary operand against
block-diagonal DFT weights, producing both channels in one PSUM bank.
Twiddle/product stages run wide ([128,512]) on DVE/GpSimd in bf16; the
final inverse stage writes the high channel to PSUM partitions 64:127
(PE tile_position), so pre/post gating runs at full 128-partition width.
DMA moves whole slabs (4 calls per tensor per slab) instead of per-channel
tiles.
"""

import math
import numpy as np
import ml_dtypes

B, D, L = 2, 768, 8192
NCORES = 8
DPC = D // NCORES          # 96 channels per core
HALF = DPC // 2            # 48; pairing (c, c+48)
SLABP = 12                 # pairs per slab
NSLAB = HALF // SLABP      # 4
NF = 2 * L                 # 16384
LOG_R_MIN, LOG_R_MAX = 0.0, 2.0

BF16 = ml_dtypes.bfloat16


def _make_consts():
    n2 = np.arange(64)
    n1 = np.arange(128)
    k1 = np.arange(128)
    k2 = np.arange(128)
    m64 = np.arange(64)

    Wc = np.exp(-2j * np.pi * np.outer(n2, k2) / 128)        # [64,128]
    T = np.exp(-2j * np.pi * np.outer(n1, k2) / NF)          # [128,128]
    W2 = np.exp(-2j * np.pi * np.outer(n1, k1) / 128)        # [128,128]
    Wcc = np.exp(+2j * np.pi * np.outer(k1, n1) / 128)       # [128,128]
    T2t = np.exp(+2j * np.pi * np.outer(k2, n1) / NF)        # [128,128]
    W2c = np.exp(+2j * np.pi * np.outer(k2, m64) / 128) / NF  # [128,64]

    bf = lambda a: np.ascontiguousarray(a, dtype=np.float32).astype(BF16)

    wblkA = np.zeros((128, 512))
    wblkB = np.zeros((128, 512))
    wblkA[0:64, 0:128] = Wc.real
    wblkA[0:64, 256:384] = Wc.imag
    wblkA[64:128, 128:256] = Wc.real
    wblkA[64:128, 384:512] = Wc.imag
    wblkB[0:64, 0:128] = -Wc.imag
    wblkB[0:64, 256:384] = Wc.real
    wblkB[64:128, 128:256] = -Wc.imag
    wblkB[64:128, 384:512] = Wc.real

    t_r2 = np.tile(T.real, (1, 2))
    t_i2 = np.tile(T.imag, (1, 2))
    t2_r2 = np.tile(T2t.real, (1, 2))
    t2_i2 = np.tile(T2t.imag, (1, 2))

    c = {}
    c["wblkA"] = bf(wblkA)
    c["wblkB"] = bf(wblkB)
    c["t_cat_a"] = bf(np.concatenate([t_r2, t_i2], axis=1))    # [128,512]
    c["t_cat_b"] = bf(np.concatenate([t_i2, t_r2], axis=1))
    c["t2_cat_a"] = bf(np.concatenate([t2_r2, t2_i2], axis=1))
    c["t2_cat_b"] = bf(np.concatenate([t2_i2, t2_r2], axis=1))
    c["w2_r"] = bf(W2.real)
    c["w2_i"] = bf(W2.imag)
    c["w2_ni"] = bf(-W2.imag)
    c["wcc_ri"] = bf(np.concatenate([Wcc.real, Wcc.imag], axis=1))    # [128,256]
    c["wcc_nir"] = bf(np.concatenate([-Wcc.imag, Wcc.real], axis=1))
    c["w2c_r"] = bf(W2c.real)       # [128,64]
    c["w2c_i"] = bf(W2c.imag)
    c["w2c_ni"] = bf(-W2c.imag)

    r = np.logspace(LOG_R_MIN, LOG_R_MAX, D).astype(np.float64)
    t = np.linspace(0.0, 1.0, L)
    decay = np.exp(-np.outer(r, t))
    c["_decay_full"] = np.ascontiguousarray(decay.astype(np.float32))
    return c


_CONSTS = _make_consts()
_NC_CACHE = {}

CONST_NAMES = ["wblkA", "wblkB", "t_cat_a", "t_cat_b", "t2_cat_a", "t2_cat_b",
               "w2_r", "w2_i", "w2_ni", "wcc_ri", "wcc_nir",
               "w2c_r", "w2c_i", "w2c_ni"]


def _build_nc():
    import concourse.bacc as bacc
    import concourse.tile as tile
    from concourse import mybir

    dt = mybir.dt
    AF = mybir.AluOpType

    nc = bacc.Bacc("TRN2", target_bir_lowering=False, debug=False,
                   num_devices=NCORES)

    def din(name, shape, d):
        return nc.dram_tensor(name, shape, d, kind="ExternalInput").ap()

    x1d = din("x1s", [B, DPC, L], dt.float32)
    x2d = din("x2s", [B, DPC, L], dt.float32)
    vd = din("vs", [B, DPC, L], dt.float32)
    hd = din("hs", [DPC, L], dt.float32)
    dbd = din("db_pair", [128, HALF], dt.float32)
    decd = din("decays", [DPC, L], dt.float32)
    cc = {}
    for nm in CONST_NAMES:
        shp = list(_CONSTS[nm].shape)
        cc[nm] = din(nm, shp, dt.bfloat16)
    outd = nc.dram_tensor("out", [B, DPC, L], dt.float32,
                          kind="ExternalOutput").ap()

    SW = SLABP * 256           # slab width for x-tensors (3072)
    KW = SLABP * 128           # slab width for h/decay (1536)

    def slab_in3(eng, t, dram, s, h, b):
        # t [128, SW]: partition (h:64)+p, col = j*256 + b*128 + q
        dst = t[h * 64:(h + 1) * 64, :].rearrange(
            "p (j b q) -> p j b q", j=SLABP, b=2, q=128)[:, :, b, :]
        src = dram[b, s * SLABP + h * HALF: s * SLABP + h * HALF + SLABP, :]
        src = src.rearrange("j (p q) -> j p q", p=64, q=128).transpose([1, 0, 2])
        eng.dma_start(dst, src)

    def slab_out3(eng, t, dram, s, h, b):
        dst = dram[b, s * SLABP + h * HALF: s * SLABP + h * HALF + SLABP, :]
        dst = dst.rearrange("j (p q) -> j p q", p=64, q=128).transpose([1, 0, 2])
        src = t[h * 64:(h + 1) * 64, :].rearrange(
            "p (j b q) -> p j b q", j=SLABP, b=2, q=128)[:, :, b, :]
        eng.dma_start(dst, src)

    def slab_in2(eng, t, dram, s, h):
        # t [128, KW]: partition (h:64)+p, col = j*128 + q
        dst = t[h * 64:(h + 1) * 64, :].rearrange(
            "p (j q) -> p j q", j=SLABP, q=128)
        src = dram[s * SLABP + h * HALF: s * SLABP + h * HALF + SLABP, :]
        src = src.rearrange("j (p q) -> j p q", p=64, q=128).transpose([1, 0, 2])
        eng.dma_start(dst, src)

    with tile.TileContext(nc, trace_sim=False) as tc:
        cpool = tc.alloc_tile_pool(name="consts", bufs=1)
        slabpool = tc.alloc_tile_pool(name="slab", bufs=2)
        fft = tc.alloc_tile_pool(name="fft", bufs=3)
        post = tc.alloc_tile_pool(name="post", bufs=4)
        ps2 = tc.alloc_tile_pool(name="ps2", bufs=2, space="PSUM")
        ps1 = tc.alloc_tile_pool(name="ps1", bufs=1, space="PSUM")

        csb = {}
        for nm, ap in cc.items():
            t = cpool.tile(list(ap.shape), dt.bfloat16, tag=nm)
            nc.sync.dma_start(t[:], ap)
            csb[nm] = t
        dbt = cpool.tile([128, HALF], dt.float32, tag="dbt")
        nc.sync.dma_start(dbt[:], dbd)

        for s in range(NSLAB):
            x2t = slabpool.tile([128, SW], dt.float32, tag="x2")
            vt = slabpool.tile([128, SW], dt.float32, tag="v")
            x1t = slabpool.tile([128, SW], dt.float32, tag="x1")
            ht = slabpool.tile([128, KW], dt.float32, tag="h")
            dct = slabpool.tile([128, KW], dt.float32, tag="dec")
            for h in range(2):
                for b in range(2):
                    slab_in3(nc.sync, x2t, x2d, s, h, b)
                    slab_in3(nc.sync, vt, vd, s, h, b)
                    slab_in3(nc.sync, x1t, x1d, s, h, b)
                slab_in2(nc.sync, ht, hd, s, h)
                slab_in2(nc.sync, dct, decd, s, h)

            ut = slabpool.tile([128, SW], dt.bfloat16, tag="u")
            nc.vector.tensor_tensor(ut[:], x2t[:], vt[:], AF.mult)
            kt = slabpool.tile([128, KW], dt.bfloat16, tag="k")
            nc.gpsimd.tensor_tensor(kt[:], ht[:], dct[:], AF.mult)

            outt = slabpool.tile([128, SW], dt.float32, tag="out")

            for j in range(SLABP):
                c = s * SLABP + j
                jc = j * 256

                # ---- S1 ----
                z0 = ps2.tile([128, 512], dt.float32, tag="z0")
                k0 = ps2.tile([128, 512], dt.float32, tag="k0")
                nc.tensor.matmul(z0[:], ut[:, jc:jc + 128], csb["wblkA"][:],
                                 start=True, stop=False)
                nc.tensor.matmul(z0[:], ut[:, jc + 128:jc + 256], csb["wblkB"][:],
                                 start=False, stop=True)
                nc.tensor.matmul(k0[:], kt[:, j * 128:(j + 1) * 128],
                                 csb["wblkA"][:], start=True, stop=True)

                # ---- forward twiddle z (DVE, fused PSUM read) ----
                ma = fft.tile([128, 512], dt.bfloat16, tag="ma")
                mb = fft.tile([128, 512], dt.bfloat16, tag="mb")
                nc.vector.tensor_tensor(ma[:], z0[:], csb["t_cat_a"][:], AF.mult)
                nc.vector.tensor_tensor(mb[:], z0[:], csb["t_cat_b"][:], AF.mult)
                z1 = fft.tile([128, 512], dt.bfloat16, tag="z1")
                nc.vector.tensor_tensor(z1[:, 0:256], ma[:, 0:256],
                                        ma[:, 256:512], AF.subtract)
                nc.vector.tensor_tensor(z1[:, 256:512], mb[:, 0:256],
                                        mb[:, 256:512], AF.add)

                # ---- forward twiddle k (scalar evac + gpsimd) ----
                k0b = fft.tile([128, 512], dt.bfloat16, tag="k0b")
                nc.scalar.copy(k0b[:], k0[:])
                kma = fft.tile([128, 512], dt.bfloat16, tag="kma")
                kmb = fft.tile([128, 512], dt.bfloat16, tag="kmb")
                nc.gpsimd.tensor_tensor(kma[:], k0b[:], csb["t_cat_a"][:], AF.mult)
                nc.gpsimd.tensor_tensor(kmb[:], k0b[:], csb["t_cat_b"][:], AF.mult)
                k1 = fft.tile([128, 512], dt.bfloat16, tag="k1")
                nc.gpsimd.tensor_tensor(k1[:, 0:256], kma[:, 0:256],
                                        kma[:, 256:512], AF.subtract)
                nc.gpsimd.tensor_tensor(k1[:, 256:512], kmb[:, 0:256],
                                        kmb[:, 256:512], AF.add)

                # ---- S2 ----
                pz = ps1.tile([128, 512], dt.float32, tag="pz")
                pk = ps1.tile([128, 512], dt.float32, tag="pk")
                nc.tensor.matmul(pz[:, 0:256], csb["w2_ni"][:], z1[:, 256:512],
                                 start=True, stop=False)
                nc.tensor.matmul(pz[:, 256:512], csb["w2_i"][:], z1[:, 0:256],
                                 start=True, stop=False)
                nc.tensor.matmul(pz[:], csb["w2_r"][:], z1[:],
                                 start=False, stop=True)
                nc.tensor.matmul(pk[:, 0:256], csb["w2_ni"][:], k1[:, 256:512],
                                 start=True, stop=False)
                nc.tensor.matmul(pk[:, 256:512], csb["w2_i"][:], k1[:, 0:256],
                                 start=True, stop=False)
                nc.tensor.matmul(pk[:], csb["w2_r"][:], k1[:],
                                 start=False, stop=True)

                # ---- spectral product (scalar evacs + DVE bf16) ----
                pzb = fft.tile([128, 512], dt.bfloat16, tag="pzb")
                pkb = fft.tile([128, 512], dt.bfloat16, tag="pkb")
                nc.scalar.copy(pzb[:], pz[:])
                nc.scalar.copy(pkb[:], pk[:])
                pa = fft.tile([128, 512], dt.bfloat16, tag="pa")
                pb = fft.tile([128, 512], dt.bfloat16, tag="pb")
                nc.vector.tensor_tensor(pa[:], pzb[:], pkb[:], AF.mult)
                nc.vector.tensor_tensor(pb[:, 0:256], pzb[:, 0:256],
                                        pkb[:, 256:512], AF.mult)
                nc.vector.tensor_tensor(pb[:, 256:512], pzb[:, 256:512],
                                        pkb[:, 0:256], AF.mult)
                py = fft.tile([128, 512], dt.bfloat16, tag="py")
                nc.vector.tensor_tensor(py[:, 0:256], pa[:, 0:256],
                                        pa[:, 256:512], AF.subtract)
                nc.vector.tensor_tensor(py[:, 256:512], pb[:, 0:256],
                                        pb[:, 256:512], AF.add)

                # ---- S1' (strided PSUM out blocks) ----
                at = ps1.tile([128, 512], dt.float32, tag="at")
                atv = at[:].rearrange("m (i c q) -> m i c q", i=2, c=2, q=128)
                for ci in range(2):
                    blocks = atv[:, :, ci, :]
                    pyr = py[:, ci * 128:(ci + 1) * 128]
                    pyi = py[:, 256 + ci * 128:256 + (ci + 1) * 128]
                    nc.tensor.matmul(blocks, pyi, csb["wcc_nir"][:],
                                     start=True, stop=False)
                    nc.tensor.matmul(blocks, pyr, csb["wcc_ri"][:],
                                     start=False, stop=True)

                # ---- inverse twiddle (DVE, fused PSUM read) ----
                ma2 = fft.tile([128, 512], dt.bfloat16, tag="ma2")
                mb2 = fft.tile([128, 512], dt.bfloat16, tag="mb2")
                nc.vector.tensor_tensor(ma2[:], at[:], csb["t2_cat_a"][:], AF.mult)
                nc.vector.tensor_tensor(mb2[:], at[:], csb["t2_cat_b"][:], AF.mult)
                bt = fft.tile([128, 512], dt.bfloat16, tag="bt")
                btv = bt[:].rearrange("p (c i q) -> p c i q", c=2, i=2, q=128)
                nc.vector.tensor_tensor(
                    btv[:, :, 0, :], ma2[:, 0:256].rearrange(
                        "p (c q) -> p c q", c=2),
                    ma2[:, 256:512].rearrange("p (c q) -> p c q", c=2),
                    AF.subtract)
                nc.vector.tensor_tensor(
                    btv[:, :, 1, :], mb2[:, 0:256].rearrange(
                        "p (c q) -> p c q", c=2),
                    mb2[:, 256:512].rearrange("p (c q) -> p c q", c=2),
                    AF.add)

                # ---- S2' (high channel to PSUM rows 64:128) ----
                yg = ps1.tile([128, 256], dt.float32, tag="yg")
                for ci in range(2):
                    rows = yg[ci * 64:(ci + 1) * 64, :]
                    btr = bt[:, ci * 256:ci * 256 + 128]
                    bti = bt[:, ci * 256 + 128:ci * 256 + 256]
                    nc.tensor.matmul(rows[:, 0:128], csb["w2c_ni"][:], bti,
                                     start=True, stop=False)
                    nc.tensor.matmul(rows[:, 128:256], csb["w2c_i"][:], btr,
                                     start=True, stop=False)
                    nc.tensor.matmul(rows[:], csb["w2c_r"][:],
                                     bt[:, ci * 256:(ci + 1) * 256],
                                     start=False, stop=True)

                # ---- post: out = (y + db*u) * x1 ----
                tt = post.tile([128, 256], dt.float32, tag="tt")
                nc.vector.scalar_tensor_tensor(
                    tt[:], ut[:, jc:jc + 256], dbt[:, c:c + 1], yg[:],
                    AF.mult, AF.add)
                nc.vector.tensor_tensor(outt[:, jc:jc + 256], tt[:],
                                        x1t[:, jc:jc + 256], AF.mult)

            for h in range(2):
                for b in range(2):
                    slab_out3(nc.sync, outt, outd, s, h, b)

        for p in (ps1, ps2, post, fft, slabpool, cpool):
            p.release()

    nc.compile()
    return nc


def _get_nc():
    if "nc" not in _NC_CACHE:
        _NC_CACHE["nc"] = _build_nc()
    return _NC_CACHE["nc"]


def make_in_maps(x1, x2, v, h, d_bias):
    c = _CONSTS
    in_maps = []
    for core in range(NCORES):
        sl = slice(core * DPC, (core + 1) * DPC)
        db = d_bias[sl]
        db_pair = np.empty((128, HALF), np.float32)
        db_pair[0:64, :] = db[None, 0:HALF]
        db_pair[64:128, :] = db[None, HALF:DPC]
        m = {
            "x1s": np.ascontiguousarray(x1[:, sl]),
            "x2s": np.ascontiguousarray(x2[:, sl]),
            "vs": np.ascontiguousarray(v[:, sl]),
            "hs": np.ascontiguousarray(h[sl]),
            "db_pair": db_pair,
            "decays": np.ascontiguousarray(c["_decay_full"][sl]),
        }
        for nm in CONST_NAMES:
            m[nm] = c[nm]
        in_maps.append(m)
    return in_maps


def kernel(x1, x2, v, h, d_bias):
    from concourse import bass_utils

    x1 = np.ascontiguousarray(x1, dtype=np.float32)
    x2 = np.ascontiguousarray(x2, dtype=np.float32)
    v = np.ascontiguousarray(v, dtype=np.float32)
    h = np.ascontiguousarray(h, dtype=np.float32)
    d_bias = np.ascontiguousarray(d_bias, dtype=np.float32)

    nc = _get_nc()
    in_maps = make_in_maps(x1, x2, v, h, d_bias)
    res = bass_utils.run_bass_kernel_spmd(
        nc, in_maps, core_ids=list(range(NCORES)))
    out = np.concatenate([r["out"] for r in res.results], axis=1)
    return out.astype(np.float32)


if __name__ == "__main__":
    rng = np.random.default_rng(0)
    inputs = {
        "x1": rng.standard_normal((B, D, L)).astype(np.float32),
        "x2": rng.standard_normal((B, D, L)).astype(np.float32),
        "v": rng.standard_normal((B, D, L)).astype(np.float32),
        "h": (rng.standard_normal((D, L)) / math.sqrt(L) * 1e-5).astype(np.float32),
        "d_bias": rng.standard_normal(D).astype(np.float32),
    }
    out = kernel(**inputs)
    print(out.shape, out.dtype)


# revision 8
# speedup vs baseline: 1.5314x; 1.0124x over previous
"""ParallelHyenaOperator Trainium2 kernel (v2: pair-stacked slab design).

out = (irfft(rfft(u,2L) * rfft(k,2L))[:L] + u*d_bias) * x1,  u = x2*v, k = h*decay

Sharding: D=768 channels split across 8 cores (96/core), no collectives.
Per core, channels are paired (c, c+48) and stacked in SBUF partitions
(c -> rows 0:64, c+48 -> rows 64:128), 12 pairs per slab, 4 slabs.
Each 16384-point FFT is a two-stage radix-128 factorization on the tensor
engine; both batches are packed as one complex series (z = u_b0 + i*u_b1).
Stage-1 matmuls take the stacked pair as the stationary operand against
block-diagonal DFT weights, producing both channels in one PSUM bank.
Twiddle/product stages run wide ([128,512]) on DVE/GpSimd in bf16; the
final inverse stage writes the high channel to PSUM partitions 64:127
(PE tile_position), so pre/post gating runs at full 128-partition width.
DMA moves whole slabs (4 calls per tensor per slab) instead of per-channel
tiles.
"""

import math
import numpy as np
import ml_dtypes

B, D, L = 2, 768, 8192
NCORES = 8
DPC = D // NCORES          # 96 channels per core
HALF = DPC // 2            # 48; pairing (c, c+48)
SLABP = 12                 # pairs per slab
NSLAB = HALF // SLABP      # 4
NF = 2 * L                 # 16384
LOG_R_MIN, LOG_R_MAX = 0.0, 2.0

BF16 = ml_dtypes.bfloat16


def _make_consts():
    n2 = np.arange(64)
    n1 = np.arange(128)
    k1 = np.arange(128)
    k2 = np.arange(128)
    m64 = np.arange(64)

    Wc = np.exp(-2j * np.pi * np.outer(n2, k2) / 128)        # [64,128]
    T = np.exp(-2j * np.pi * np.outer(n1, k2) / NF)          # [128,128]
    W2 = np.exp(-2j * np.pi * np.outer(n1, k1) / 128)        # [128,128]
    Wcc = np.exp(+2j * np.pi * np.outer(k1, n1) / 128)       # [128,128]
    T2t = np.exp(+2j * np.pi * np.outer(k2, n1) / NF)        # [128,128]
    W2c = np.exp(+2j * np.pi * np.outer(k2, m64) / 128) / NF  # [128,64]

    bf = lambda a: np.ascontiguousarray(a, dtype=np.float32).astype(BF16)

    wblkA = np.zeros((128, 512))
    wblkB = np.zeros((128, 512))
    wblkA[0:64, 0:128] = Wc.real
    wblkA[0:64, 256:384] = Wc.imag
    wblkA[64:128, 128:256] = Wc.real
    wblkA[64:128, 384:512] = Wc.imag
    wblkB[0:64, 0:128] = -Wc.imag
    wblkB[0:64, 256:384] = Wc.real
    wblkB[64:128, 128:256] = -Wc.imag
    wblkB[64:128, 384:512] = Wc.real

    t_r2 = np.tile(T.real, (1, 2))
    t_i2 = np.tile(T.imag, (1, 2))
    t2_r2 = np.tile(T2t.real, (1, 2))
    t2_i2 = np.tile(T2t.imag, (1, 2))

    c = {}
    c["wblkA"] = bf(wblkA)
    c["wblkB"] = bf(wblkB)
    c["t_cat_a"] = bf(np.concatenate([t_r2, t_i2], axis=1))    # [128,512]
    c["t_cat_b"] = bf(np.concatenate([t_i2, t_r2], axis=1))
    c["t2_cat_a"] = bf(np.concatenate([t2_r2, t2_i2], axis=1))
    c["t2_cat_b"] = bf(np.concatenate([t2_i2, t2_r2], axis=1))
    c["w2_r"] = bf(W2.real)
    c["w2_i"] = bf(W2.imag)
    c["w2_ni"] = bf(-W2.imag)
    c["wcc_ri"] = bf(np.concatenate([Wcc.real, Wcc.imag], axis=1))    # [128,256]
    c["wcc_nir"] = bf(np.concatenate([-Wcc.imag, Wcc.real], axis=1))
    c["w2c_r"] = bf(W2c.real)       # [128,64]
    c["w2c_i"] = bf(W2c.imag)
    c["w2c_ni"] = bf(-W2c.imag)

    r = np.logspace(LOG_R_MIN, LOG_R_MAX, D).astype(np.float64)
    t = np.linspace(0.0, 1.0, L)
    decay = np.exp(-np.outer(r, t))
    c["_decay_full"] = np.ascontiguousarray(decay.astype(np.float32))
    return c


_CONSTS = _make_consts()
_NC_CACHE = {}

CONST_NAMES = ["wblkA", "wblkB", "t_cat_a", "t_cat_b", "t2_cat_a", "t2_cat_b",
               "w2_r", "w2_i", "w2_ni", "wcc_ri", "wcc_nir",
               "w2c_r", "w2c_i", "w2c_ni"]


def _build_nc():
    import concourse.bacc as bacc
    import concourse.tile as tile
    from concourse import mybir

    dt = mybir.dt
    AF = mybir.AluOpType

    nc = bacc.Bacc("TRN2", target_bir_lowering=False, debug=False,
                   num_devices=NCORES)

    def din(name, shape, d):
        return nc.dram_tensor(name, shape, d, kind="ExternalInput").ap()

    x1d = din("x1s", [B, DPC, L], dt.float32)
    x2d = din("x2s", [B, DPC, L], dt.float32)
    vd = din("vs", [B, DPC, L], dt.float32)
    hd = din("hs", [DPC, L], dt.float32)
    dbd = din("db_pair", [128, HALF], dt.float32)
    decd = din("decays", [DPC, L], dt.float32)
    cc = {}
    for nm in CONST_NAMES:
        shp = list(_CONSTS[nm].shape)
        cc[nm] = din(nm, shp, dt.bfloat16)
    outd = nc.dram_tensor("out", [B, DPC, L], dt.float32,
                          kind="ExternalOutput").ap()

    SW = SLABP * 256           # slab width for x-tensors (3072)
    KW = SLABP * 128           # slab width for h/decay (1536)

    def slab_in3(eng, t, dram, s, h, b):
        # t [128, SW]: partition (h:64)+p, col = j*256 + b*128 + q
        dst = t[h * 64:(h + 1) * 64, :].rearrange(
            "p (j b q) -> p j b q", j=SLABP, b=2, q=128)[:, :, b, :]
        src = dram[b, s * SLABP + h * HALF: s * SLABP + h * HALF + SLABP, :]
        src = src.rearrange("j (p q) -> j p q", p=64, q=128).transpose([1, 0, 2])
        eng.dma_start(dst, src)

    def slab_out3(eng, t, dram, s, h, b):
        dst = dram[b, s * SLABP + h * HALF: s * SLABP + h * HALF + SLABP, :]
        dst = dst.rearrange("j (p q) -> j p q", p=64, q=128).transpose([1, 0, 2])
        src = t[h * 64:(h + 1) * 64, :].rearrange(
            "p (j b q) -> p j b q", j=SLABP, b=2, q=128)[:, :, b, :]
        eng.dma_start(dst, src)

    def slab_in2(eng, t, dram, s, h):
        # t [128, KW]: partition (h:64)+p, col = j*128 + q
        dst = t[h * 64:(h + 1) * 64, :].rearrange(
            "p (j q) -> p j q", j=SLABP, q=128)
        src = dram[s * SLABP + h * HALF: s * SLABP + h * HALF + SLABP, :]
        src = src.rearrange("j (p q) -> j p q", p=64, q=128).transpose([1, 0, 2])
        eng.dma_start(dst, src)

    with tile.TileContext(nc, trace_sim=False) as tc:
        cpool = tc.alloc_tile_pool(name="consts", bufs=1)
        slabpool = tc.alloc_tile_pool(name="slab", bufs=2)
        fft = tc.alloc_tile_pool(name="fft", bufs=3)
        post = tc.alloc_tile_pool(name="post", bufs=4)
        ps2 = tc.alloc_tile_pool(name="ps2", bufs=2, space="PSUM")
        ps1 = tc.alloc_tile_pool(name="ps1", bufs=1, space="PSUM")

        csb = {}
        for nm, ap in cc.items():
            t = cpool.tile(list(ap.shape), dt.bfloat16, tag=nm)
            nc.sync.dma_start(t[:], ap)
            csb[nm] = t
        dbt = cpool.tile([128, HALF], dt.float32, tag="dbt")
        nc.sync.dma_start(dbt[:], dbd)

        for s in range(NSLAB):
            x2t = slabpool.tile([128, SW], dt.float32, tag="x2")
            vt = slabpool.tile([128, SW], dt.float32, tag="v")
            x1t = slabpool.tile([128, SW], dt.float32, tag="x1")
            ht = slabpool.tile([128, KW], dt.float32, tag="h")
            dct = slabpool.tile([128, KW], dt.float32, tag="dec")
            for h in range(2):
                for b in range(2):
                    slab_in3(nc.sync, x2t, x2d, s, h, b)
                    slab_in3(nc.sync, vt, vd, s, h, b)
                    slab_in3(nc.sync, x1t, x1d, s, h, b)
                slab_in2(nc.scalar, ht, hd, s, h)
                slab_in2(nc.scalar, dct, decd, s, h)

            ut = slabpool.tile([128, SW], dt.bfloat16, tag="u")
            nc.vector.tensor_tensor(ut[:], x2t[:], vt[:], AF.mult)
            kt = slabpool.tile([128, KW], dt.bfloat16, tag="k")
            nc.gpsimd.tensor_tensor(kt[:], ht[:], dct[:], AF.mult)

            outt = slabpool.tile([128, SW], dt.float32, tag="out")

            for j in range(SLABP):
                c = s * SLABP + j
                jc = j * 256

                # ---- S1 ----
                z0 = ps2.tile([128, 512], dt.float32, tag="z0")
                k0 = ps2.tile([128, 512], dt.float32, tag="k0")
                nc.tensor.matmul(z0[:], ut[:, jc:jc + 128], csb["wblkA"][:],
                                 start=True, stop=False)
                nc.tensor.matmul(z0[:], ut[:, jc + 128:jc + 256], csb["wblkB"][:],
                                 start=False, stop=True)
                nc.tensor.matmul(k0[:], kt[:, j * 128:(j + 1) * 128],
                                 csb["wblkA"][:], start=True, stop=True)

                # ---- forward twiddle z (scalar evac + DVE bf16) ----
                z0b = fft.tile([128, 512], dt.bfloat16, tag="z0b")
                nc.scalar.copy(z0b[:], z0[:])
                ma = fft.tile([128, 512], dt.bfloat16, tag="ma")
                mb = fft.tile([128, 512], dt.bfloat16, tag="mb")
                nc.vector.tensor_tensor(ma[:], z0b[:], csb["t_cat_a"][:], AF.mult)
                nc.vector.tensor_tensor(mb[:], z0b[:], csb["t_cat_b"][:], AF.mult)
                z1 = fft.tile([128, 512], dt.bfloat16, tag="z1")
                nc.vector.tensor_tensor(z1[:, 0:256], ma[:, 0:256],
                                        ma[:, 256:512], AF.subtract)
                nc.vector.tensor_tensor(z1[:, 256:512], mb[:, 0:256],
                                        mb[:, 256:512], AF.add)

                # ---- forward twiddle k (scalar evac + DVE bf16) ----
                k0b = fft.tile([128, 512], dt.bfloat16, tag="k0b")
                nc.scalar.copy(k0b[:], k0[:])
                kma = fft.tile([128, 512], dt.bfloat16, tag="kma")
                kmb = fft.tile([128, 512], dt.bfloat16, tag="kmb")
                nc.vector.tensor_tensor(kma[:], k0b[:], csb["t_cat_a"][:], AF.mult)
                nc.vector.tensor_tensor(kmb[:], k0b[:], csb["t_cat_b"][:], AF.mult)
                k1 = fft.tile([128, 512], dt.bfloat16, tag="k1")
                nc.vector.tensor_tensor(k1[:, 0:256], kma[:, 0:256],
                                        kma[:, 256:512], AF.subtract)
                nc.vector.tensor_tensor(k1[:, 256:512], kmb[:, 0:256],
                                        kmb[:, 256:512], AF.add)

                # ---- S2 ----
                pz = ps1.tile([128, 512], dt.float32, tag="pz")
                pk = ps1.tile([128, 512], dt.float32, tag="pk")
                nc.tensor.matmul(pz[:, 0:256], csb["w2_ni"][:], z1[:, 256:512],
                                 start=True, stop=False)
                nc.tensor.matmul(pz[:, 256:512], csb["w2_i"][:], z1[:, 0:256],
                                 start=True, stop=False)
                nc.tensor.matmul(pz[:], csb["w2_r"][:], z1[:],
                                 start=False, stop=True)
                nc.tensor.matmul(pk[:, 0:256], csb["w2_ni"][:], k1[:, 256:512],
                                 start=True, stop=False)
                nc.tensor.matmul(pk[:, 256:512], csb["w2_i"][:], k1[:, 0:256],
                                 start=True, stop=False)
                nc.tensor.matmul(pk[:], csb["w2_r"][:], k1[:],
                                 start=False, stop=True)

                # ---- spectral product (scalar evacs + DVE bf16) ----
                pzb = fft.tile([128, 512], dt.bfloat16, tag="pzb")
                pkb = fft.tile([128, 512], dt.bfloat16, tag="pkb")
                nc.scalar.copy(pzb[:], pz[:])
                nc.scalar.copy(pkb[:], pk[:])
                pa = fft.tile([128, 512], dt.bfloat16, tag="pa")
                pb = fft.tile([128, 512], dt.bfloat16, tag="pb")
                nc.gpsimd.tensor_tensor(pa[:], pzb[:], pkb[:], AF.mult)
                nc.gpsimd.tensor_tensor(pb[:, 0:256], pzb[:, 0:256],
                                        pkb[:, 256:512], AF.mult)
                nc.gpsimd.tensor_tensor(pb[:, 256:512], pzb[:, 256:512],
                                        pkb[:, 0:256], AF.mult)
                py = fft.tile([128, 512], dt.bfloat16, tag="py")
                nc.gpsimd.tensor_tensor(py[:, 0:256], pa[:, 0:256],
                                        pa[:, 256:512], AF.subtract)
                nc.gpsimd.tensor_tensor(py[:, 256:512], pb[:, 0:256],
                                        pb[:, 256:512], AF.add)

                # ---- S1' (strided PSUM out blocks) ----
                at = ps1.tile([128, 512], dt.float32, tag="at")
                atv = at[:].rearrange("m (i c q) -> m i c q", i=2, c=2, q=128)
                for ci in range(2):
                    blocks = atv[:, :, ci, :]
                    pyr = py[:, ci * 128:(ci + 1) * 128]
                    pyi = py[:, 256 + ci * 128:256 + (ci + 1) * 128]
                    nc.tensor.matmul(blocks, pyi, csb["wcc_nir"][:],
                                     start=True, stop=False)
                    nc.tensor.matmul(blocks, pyr, csb["wcc_ri"][:],
                                     start=False, stop=True)

                # ---- inverse twiddle (scalar evac + DVE bf16) ----
                atb = fft.tile([128, 512], dt.bfloat16, tag="atb")
                nc.scalar.copy(atb[:], at[:])
                ma2 = fft.tile([128, 512], dt.bfloat16, tag="ma2")
                mb2 = fft.tile([128, 512], dt.bfloat16, tag="mb2")
                nc.vector.tensor_tensor(ma2[:], atb[:], csb["t2_cat_a"][:], AF.mult)
                nc.vector.tensor_tensor(mb2[:], atb[:], csb["t2_cat_b"][:], AF.mult)
                bt = fft.tile([128, 512], dt.bfloat16, tag="bt")
                btv = bt[:].rearrange("p (c i q) -> p c i q", c=2, i=2, q=128)
                nc.vector.tensor_tensor(
                    btv[:, :, 0, :], ma2[:, 0:256].rearrange(
                        "p (c q) -> p c q", c=2),
                    ma2[:, 256:512].rearrange("p (c q) -> p c q", c=2),
                    AF.subtract)
                nc.vector.tensor_tensor(
                    btv[:, :, 1, :], mb2[:, 0:256].rearrange(
                        "p (c q) -> p c q", c=2),
                    mb2[:, 256:512].rearrange("p (c q) -> p c q", c=2),
                    AF.add)

                # ---- S2' (high channel to PSUM rows 64:128) ----
                yg = ps1.tile([128, 256], dt.float32, tag="yg")
                for ci in range(2):
                    rows = yg[ci * 64:(ci + 1) * 64, :]
                    btr = bt[:, ci * 256:ci * 256 + 128]
                    bti = bt[:, ci * 256 + 128:ci * 256 + 256]
                    nc.tensor.matmul(rows[:, 0:128], csb["w2c_ni"][:], bti,
                                     start=True, stop=False)
                    nc.tensor.matmul(rows[:, 128:256], csb["w2c_i"][:], btr,
                                     start=True, stop=False)
                    nc.tensor.matmul(rows[:], csb["w2c_r"][:],
                                     bt[:, ci * 256:(ci + 1) * 256],
                                     start=False, stop=True)

                # ---- post: out = (y + db*u) * x1 ----
                ygb = post.tile([128, 256], dt.bfloat16, tag="ygb")
                nc.scalar.copy(ygb[:], yg[:])
                tt = post.tile([128, 256], dt.bfloat16, tag="tt")
                nc.vector.scalar_tensor_tensor(
                    tt[:], ut[:, jc:jc + 256], dbt[:, c:c + 1], ygb[:],
                    AF.mult, AF.add)
                nc.vector.tensor_tensor(outt[:, jc:jc + 256], tt[:],
                                        x1t[:, jc:jc + 256], AF.mult)

            for h in range(2):
                for b in range(2):
                    slab_out3(nc.scalar, outt, outd, s, h, b)

        for p in (ps1, ps2, post, fft, slabpool, cpool):
            p.release()

    nc.compile()
    return nc


def _get_nc():
    if "nc" not in _NC_CACHE:
        _NC_CACHE["nc"] = _build_nc()
    return _NC_CACHE["nc"]


def make_in_maps(x1, x2, v, h, d_bias):
    c = _CONSTS
    in_maps = []
    for core in range(NCORES):
        sl = slice(core * DPC, (core + 1) * DPC)
        db = d_bias[sl]
        db_pair = np.empty((128, HALF), np.float32)
        db_pair[0:64, :] = db[None, 0:HALF]
        db_pair[64:128, :] = db[None, HALF:DPC]
        m = {
            "x1s": np.ascontiguousarray(x1[:, sl]),
            "x2s": np.ascontiguousarray(x2[:, sl]),
            "vs": np.ascontiguousarray(v[:, sl]),
            "hs": np.ascontiguousarray(h[sl]),
            "db_pair": db_pair,
            "decays": np.ascontiguousarray(c["_decay_full"][sl]),
        }
        for nm in CONST_NAMES:
            m[nm] = c[nm]
        in_maps.append(m)
    return in_maps


def kernel(x1, x2, v, h, d_bias):
    from concourse import bass_utils

    x1 = np.ascontiguousarray(x1, dtype=np.float32)
    x2 = np.ascontiguousarray(x2, dtype=np.float32)
    v = np.ascontiguousarray(v, dtype=np.float32)
    h = np.ascontiguousarray(h, dtype=np.float32)
    d_bias = np.ascontiguousarray(d_bias, dtype=np.float32)

    nc = _get_nc()
    in_maps = make_in_maps(x1, x2, v, h, d_bias)
    res = bass_utils.run_bass_kernel_spmd(
        nc, in_maps, core_ids=list(range(NCORES)))
    out = np.concatenate([r["out"] for r in res.results], axis=1)
    return out.astype(np.float32)


if __name__ == "__main__":
    rng = np.random.default_rng(0)
    inputs = {
        "x1": rng.standard_normal((B, D, L)).astype(np.float32),
        "x2": rng.standard_normal((B, D, L)).astype(np.float32),
        "v": rng.standard_normal((B, D, L)).astype(np.float32),
        "h": (rng.standard_normal((D, L)) / math.sqrt(L) * 1e-5).astype(np.float32),
        "d_bias": rng.standard_normal(D).astype(np.float32),
    }
    out = kernel(**inputs)
    print(out.shape, out.dtype)
